# revision 7
# baseline (speedup 1.0000x reference)
"""Causal self-attention (B=2, S=4096, D=512, H=8) on 8 Trainium2 cores.

Sharding: core c handles batch b = c//4 and heads {2*(c%4), 2*(c%4)+1}.

Fused single-pipeline design (v2): per query-chunk J the kernel runs
attention for head0 then head1 (k-major transposed scores, exp on ACT with
the padding mask folded into the per-partition bias), while the PE slack
under the ACT-bound steady state absorbs interleaved "filler" work: the
q/k/v projections for chunk J+1, the V transposes, and the q-major output
projection for chunk J-1.  Scores PSUM is double-buffered so ACT never
waits on QK; PV accumulates numerators + softmax denominator (ones column)
per head; the Pool engine moves PV results and output-projection tiles out
of PSUM (bf16), keeping DVE/ACT free.  Outputs are per-head undivided
projections po_h [S, 512] bf16 plus denominators; the host divides, sums
heads/cores, and adds bo.

PSUM map (8 banks): st 2 bufs x [128,1024]f32 (4) | pv [65,1024]f32 (2)
| aux 2 bufs x [128,512]f32 shared by proj pieces / V transposes / outproj.

Head x row-group layout: qT/kT keep head0 on partitions 0-63, head1 on
64-127; dupq/dupk hold the swapped copy so head h can issue even kb blocks
on PE row group 0 and odd kb blocks on row group 64 (concurrent tiles).
"""

import sys

sys.path.insert(0, "/opt/trn_rl_repo")

from contextlib import ExitStack

import ml_dtypes
import numpy as np

import concourse.bass as bass
import concourse.tile as tile
from concourse import bacc, bass_utils, mybir

B, S, D = 2, 4096, 512
H, HD = 8, 64
NCORES = 8
F32 = mybir.dt.float32
BF16 = mybir.dt.bfloat16
EXP = mybir.ActivationFunctionType.Exp
NPBF16 = ml_dtypes.bfloat16

CHUNK = 1024                  # query-chunk width
NCHUNK = S // CHUNK           # 4
KBLK = 128                    # key block (partition dim)
KB_PER_CHUNK = CHUNK // KBLK  # 8
NEG = -1.0e30


def _pieces(col0):
    """Split [col0, CHUNK) into <=512-wide pieces aligned to 512 boundaries."""
    out = []
    c = col0
    while c < CHUNK:
        nxt = min(CHUNK, (c // 512 + 1) * 512)
        out.append((c, nxt))
        c = nxt
    return out


class _Emitter:
    def __init__(self, nc, tc, ctx, io):
        self.nc = nc
        (self.xT, self.wq_p, self.wk_p, self.wv_p, self.wo01, self.bqkv,
         self.kbias, self.trimask, self.ident2, self.po0, self.po1,
         self.dens) = io

        const = ctx.enter_context(tc.tile_pool(name="const", bufs=1))
        self.sb = ctx.enter_context(tc.tile_pool(name="sb", bufs=1))
        self.etp = ctx.enter_context(tc.tile_pool(name="etp", bufs=6))
        self.xp = ctx.enter_context(tc.tile_pool(name="xp", bufs=2))
        self.ps_st = ctx.enter_context(
            tc.tile_pool(name="ps_st", bufs=2, space="PSUM"))
        self.ps_pv = ctx.enter_context(
            tc.tile_pool(name="ps_pv", bufs=1, space="PSUM"))
        self.ps_aux = ctx.enter_context(
            tc.tile_pool(name="ps_aux", bufs=2, space="PSUM"))

        # constants / weights
        self.wq_sb = const.tile([128, 512], BF16, tag="wq")
        self.wk_sb = const.tile([128, 512], BF16, tag="wk")
        self.wv_sb = const.tile([128, 512], BF16, tag="wv")
        self.wo_sb = const.tile([128, 512], BF16, tag="wo")
        self.bqkv_sb = const.tile([128, 3], F32, tag="bqkv")
        self.kbias_sb = const.tile([128, 32], F32, tag="kbias")
        self.tri_sb = const.tile([128, 128], BF16, tag="tri")
        self.id2_sb = const.tile([128, 64], BF16, tag="id2")
        onesf_sb = const.tile([128, 1], F32, tag="onesf")
        nc.vector.memset(onesf_sb[:], 1.0)
        for t, a in ((self.wq_sb, self.wq_p), (self.wk_sb, self.wk_p),
                     (self.wv_sb, self.wv_p), (self.bqkv_sb, self.bqkv),
                     (self.id2_sb, self.ident2), (self.kbias_sb, self.kbias),
                     (self.tri_sb, self.trimask), (self.wo_sb, self.wo01)):
            nc.sync.dma_start(t[:], a[:])

        # persistent intermediates
        self.qT = self.sb.tile([128, S], BF16, tag="qT")
        self.kT = self.sb.tile([128, S], BF16, tag="kT")
        self.dupq = self.sb.tile([128, S], BF16, tag="dupq")
        self.dupk = self.sb.tile([128, S], BF16, tag="dupk")
        self.v0 = self.sb.tile([128, 32 * 65], BF16, tag="v0")
        self.v1 = self.sb.tile([128, 32 * 65], BF16, tag="v1")
        self.oT01 = self.sb.tile([128, S], BF16, tag="oT01")
        self.den0 = self.sb.tile([1, S], F32, tag="den0")
        self.den1 = self.sb.tile([1, S], F32, tag="den1")
        for vdst in (self.v0, self.v1):
            ones_col = vdst.rearrange("p (k c) -> p k c", c=65)[:, :, 64:65]
            nc.vector.tensor_copy(
                ones_col, onesf_sb[:].to_broadcast((128, 32, 1)))

        self._fill = []

    # ---------------- filler units (emitted between attention steps) ----
    def fill(self, n=1):
        for _ in range(n):
            if self._fill:
                self._fill.pop(0)()

    def flush_fill(self):
        while self._fill:
            self._fill.pop(0)()

    def emit_dma_x(self, J):
        """Queue DMA of x chunk J. Returns the x tiles."""
        nc = self.nc
        x_sb = []
        for ks in range(4):
            xt = self.xp.tile([128, CHUNK], BF16, tag=f"x{ks}")
            x_sb.append(xt)

        def unit(ks=None):
            for ks in range(4):
                nc.sync.dma_start(
                    x_sb[ks][:],
                    self.xT[ks * 128:(ks + 1) * 128,
                            J * CHUNK:(J + 1) * CHUNK])
        return x_sb, unit

    def proj_units(self, J, x_sb):
        """q/k/v projections for chunk J + V transpose/repack + dup copies."""
        nc = self.nc
        units = []
        vT = self.sb.tile([128, CHUNK], BF16, tag="vTc", bufs=2)
        csl = slice(J * CHUNK, (J + 1) * CHUNK)

        for w_sb, bcol, dest, base in (
                (self.wq_sb, 0, self.qT, J * CHUNK),
                (self.wk_sb, 1, self.kT, J * CHUNK),
                (self.wv_sb, 2, vT, 0)):
            for half in range(2):
                lo = half * 512

                def unit(w_sb=w_sb, bcol=bcol, dest=dest, base=base, lo=lo):
                    ps = self.ps_aux.tile([128, 512], F32, tag="aux")
                    for ks in range(4):
                        nc.tensor.matmul(
                            ps[:],
                            w_sb[:, ks * 128:(ks + 1) * 128],
                            x_sb[ks][:, lo:lo + 512],
                            start=(ks == 0), stop=(ks == 3))
                    nc.vector.tensor_scalar_add(
                        dest[:, base + lo:base + lo + 512], ps[:],
                        self.bqkv_sb[:, bcol:bcol + 1])
                units.append(unit)

        # V -> k-major 65-stride blocks (per head)
        for hh, vdst in ((0, self.v0), (1, self.v1)):
            def unit(hh=hh, vdst=vdst):
                tr = self.ps_aux.tile([128, 512], BF16, tag="aux")
                for i in range(8):
                    kb = J * 8 + i
                    nc.tensor.transpose(
                        tr[:, i * 64:(i + 1) * 64],
                        vT[hh * 64:(hh + 1) * 64,
                           (kb - J * 8) * KBLK:(kb - J * 8 + 1) * KBLK],
                        self.id2_sb[hh * 64:(hh + 1) * 64, :])
                dst = vdst[:, J * 8 * 65:(J + 1) * 8 * 65]
                dst = dst.rearrange("p (k c) -> p k c", c=65)[:, :, 0:64]
                nc.vector.tensor_copy(
                    dst, tr.rearrange("p (k c) -> p k c", c=64))
            units.append(unit)

        def dup_unit():
            # swapped copies: dupX rows 64-127 <- head0, rows 0-63 <- head1
            # (Pool engine: SBUF-to-SBUF, keeps DVE free)
            nc.gpsimd.tensor_copy(self.dupq[64:128, csl], self.qT[0:64, csl])
            nc.gpsimd.tensor_copy(self.dupq[0:64, csl], self.qT[64:128, csl])
            nc.gpsimd.tensor_copy(self.dupk[64:128, csl], self.kT[0:64, csl])
            nc.gpsimd.tensor_copy(self.dupk[0:64, csl], self.kT[64:128, csl])
        units.append(dup_unit)
        return units

    def ph3_units(self, J):
        """Output projection for chunk J (q-major), per 128-query block."""
        nc = self.nc
        units = []
        for qb in range(8):
            def unit(qb=qb):
                q0 = J * CHUNK + qb * 128
                pos = []
                for hh in range(2):
                    hsl = slice(hh * 64, (hh + 1) * 64)
                    po = self.ps_aux.tile([128, 512], F32, tag="aux")
                    nc.tensor.matmul(
                        po[:], self.oT01[hsl, q0:q0 + 128],
                        self.wo_sb[hsl, :], start=True, stop=True)
                    pos.append(po)
                for hh, po in enumerate(pos):
                    posb = self.sb.tile([128, 512], BF16, tag="posb", bufs=4)
                    nc.vector.tensor_copy(posb[:], po[:])
                    dram = self.po0 if hh == 0 else self.po1
                    nc.sync.dma_start(dram[q0:q0 + 128, :], posb[:])
            units.append(unit)
        return units

    # ---------------- attention for (chunk J, head hh) -------------------
    def attention(self, J, hh):
        nc = self.nc
        # row-group assignment: "normal" tiles serve head hh at group
        # hh*64; the swapped dup tiles serve it at the other group.
        qn, kn = (self.qT, self.kT)
        qs, ks_ = (self.dupq, self.dupk)
        vsb = self.v0 if hh == 0 else self.v1
        den = self.den0 if hh == 0 else self.den1
        hs0 = hh * 64            # group of the natural layout
        hs1 = 64 - hs0           # group of the swapped layout

        pv = self.ps_pv.tile([65, CHUNK], F32, tag="pv")
        nkb = KB_PER_CHUNK * (J + 1)
        csl = slice(J * CHUNK, (J + 1) * CHUNK)

        def emit_qk(kb):
            p = kb - KB_PER_CHUNK * J
            col0 = KBLK * p if p >= 0 else 0
            if kb % 2 == 0:
                qsrc, ksrc, g = qn, kn, hs0
            else:
                qsrc, ksrc, g = qs, ks_, hs1
            gs = slice(g, g + 64)
            st = self.ps_st.tile([128, CHUNK], F32, tag="st")
            for (a, b) in _pieces(col0):
                nc.tensor.matmul(
                    st[:, a:b],
                    ksrc[gs, kb * KBLK:(kb + 1) * KBLK],
                    qsrc[gs, J * CHUNK + a:J * CHUNK + b],
                    start=True, stop=True)
            return st

        # pipelined: QK for kb+1 issues (PE) before PV for kb, so the PE
        # keeps ACT fed while PV waits on the current exp.
        st = emit_qk(0)
        for kb in range(nkb):
            p = kb - KB_PER_CHUNK * J
            col0 = KBLK * p if p >= 0 else 0
            pieces = _pieces(col0)
            et = self.etp.tile([128, CHUNK], BF16, tag="et")
            nc.scalar.activation(
                et[:, col0:], st[:, col0:], EXP,
                bias=self.kbias_sb[:, kb:kb + 1], scale=0.125)
            if p >= 0:
                nc.vector.tensor_mul(
                    et[:, col0:col0 + KBLK], et[:, col0:col0 + KBLK],
                    self.tri_sb[:])
            if kb + 1 < nkb:
                st = emit_qk(kb + 1)
            self.fill(self._rate)
            for (a, b) in pieces:
                last_a = (kb == KB_PER_CHUNK * J + 3 and a < 512)
                last_b = (kb == nkb - 1)
                nc.tensor.matmul(
                    pv[:, a:b],
                    vsb[:, kb * 65:(kb + 1) * 65],
                    et[:, a:b],
                    start=(kb == 0),
                    stop=(last_a if a < 512 else last_b))
        # move pv out (DVE; GPSIMD cannot read PSUM), freeing the pv slot
        hsl = slice(hh * 64, (hh + 1) * 64)
        nc.vector.tensor_copy(self.oT01[hsl, csl], pv[0:64, :])
        nc.vector.tensor_copy(den[:, csl], pv[64:65, :])

    def run(self):
        nc = self.nc
        # prologue: chunk 0 inputs + projections, serial
        x0, dma0 = self.emit_dma_x(0)
        dma0()
        for u in self.proj_units(0, x0):
            u()
        x1, dma1 = self.emit_dma_x(1)
        dma1()

        xs = {1: x1}
        for J in range(NCHUNK):
            # build filler list for this chunk
            fills = []
            if J + 1 < NCHUNK:
                fills += self.proj_units(J + 1, xs[J + 1])
            if J + 2 < NCHUNK:
                xn, dman = self.emit_dma_x(J + 2)
                xs[J + 2] = xn
                fills.append(dman)
            if J > 0:
                fills += self.ph3_units(J - 1)
            self._fill = fills
            niter = 2 * KB_PER_CHUNK * (J + 1)
            self._rate = (len(fills) + niter - 1) // niter if fills else 0
            if self._rate == 0 and fills:
                self._rate = 1
            self.attention(J, 0)
            self.attention(J, 1)
            self.flush_fill()

        for u in self.ph3_units(NCHUNK - 1):
            u()
        nc.sync.dma_start(self.dens[0:1, :], self.den0[:])
        nc.sync.dma_start(self.dens[1:2, :], self.den1[:])


def _emit(nc, tc, ctx, io):
    _Emitter(nc, tc, ctx, io).run()


_CACHED = None


def _build():
    global _CACHED
    if _CACHED is not None:
        return _CACHED
    nc = bacc.Bacc("TRN2", target_bir_lowering=False, debug=False,
                   enable_asserts=False, num_devices=NCORES)
    names = [
        ("xT", [D, S], BF16), ("wq_p", [128, 512], BF16),
        ("wk_p", [128, 512], BF16), ("wv_p", [128, 512], BF16),
        ("wo01", [128, 512], BF16),
        ("bqkv", [128, 3], F32), ("kbias", [128, 32], F32),
        ("trimask", [128, 128], BF16), ("ident2", [128, 64], BF16),
    ]
    aps = [nc.dram_tensor(n, sh, dt_, kind="ExternalInput").ap()
           for n, sh, dt_ in names]
    po0 = nc.dram_tensor("po0", [S, D], BF16, kind="ExternalOutput").ap()
    po1 = nc.dram_tensor("po1", [S, D], BF16, kind="ExternalOutput").ap()
    dens = nc.dram_tensor("dens", [2, S], F32, kind="ExternalOutput").ap()
    with tile.TileContext(nc) as tc, ExitStack() as ctx:
        _emit(nc, tc, ctx, aps + [po0, po1, dens])
    nc.compile()
    _CACHED = nc
    return nc


def _host_inputs(x, attention_mask, Wq, bq, Wk, bk, Wv, bv, Wo, bo):
    f = np.float32
    x = np.asarray(x, f)
    mask = np.asarray(attention_mask)
    Wq, Wk, Wv, Wo = (np.asarray(w, f) for w in (Wq, Wk, Wv, Wo))
    bq, bk, bv = (np.asarray(b_, f) for b_ in (bq, bk, bv))
    tri = np.triu(np.ones((128, 128), NPBF16))      # [k,q]: 1 where q >= k
    id2 = np.tile(np.eye(64, dtype=NPBF16), (2, 1))
    in_maps = []
    for c in range(NCORES):
        b = c // 4
        h0 = 2 * (c % 4)
        hsl = slice(64 * h0, 64 * h0 + 128)

        def pack_w(W):
            wt = W[hsl, :].T                        # [512, 128] = Wh^T
            return np.ascontiguousarray(
                wt.reshape(4, 128, 128).transpose(1, 0, 2)
                .reshape(128, 512).astype(NPBF16))

        wo_t = Wo[:, hsl].T.astype(NPBF16)           # [128, 512]
        kb = np.where(mask[b] != 0, f(0.0), f(NEG)).astype(f)
        in_maps.append({
            "xT": np.ascontiguousarray(x[b].T.astype(NPBF16)),
            "wq_p": pack_w(Wq), "wk_p": pack_w(Wk), "wv_p": pack_w(Wv),
            "wo01": np.ascontiguousarray(wo_t),
            "bqkv": np.ascontiguousarray(
                np.stack([bq[hsl], bk[hsl], bv[hsl]], axis=1)),
            "kbias": np.ascontiguousarray(kb.reshape(32, 128).T),
            "trimask": tri, "ident2": id2,
        })
    return in_maps


def _assemble(results, bo):
    out = np.zeros((B, S, D), np.float32)
    for c in range(NCORES):
        r = results[c]
        dens = r["dens"]
        part = (r["po0"].astype(np.float32) / dens[0][:, None]
                + r["po1"].astype(np.float32) / dens[1][:, None])
        out[c // 4] += part
    out += np.asarray(bo, np.float32)
    return out


def kernel(**inputs) -> np.ndarray:
    nc = _build()
    in_maps = _host_inputs(**inputs)
    last_err = None
    for attempt in range(3):
        try:
            res = bass_utils.run_bass_kernel_spmd(
                nc, in_maps, core_ids=list(range(NCORES)))
            out = _assemble(res.results, inputs["bo"])
        except Exception as e:  # transient NRT/axon device errors
            last_err = e
            continue
        if np.isfinite(out).all():
            return out
        last_err = RuntimeError("non-finite output")
    raise last_err


def run_traced(inputs, **kwargs):
    """test.py helper: run with NTFF tracing, return (out, BassKernelResults)."""
    nc = _build()
    in_maps = _host_inputs(**inputs)
    res = bass_utils.run_bass_kernel_spmd(
        nc, in_maps, core_ids=list(range(NCORES)), trace=True, **kwargs)
    return _assemble(res.results, inputs["bo"]), res


# revision 14
# speedup vs baseline: 1.3093x; 1.3093x over previous
"""Causal self-attention (B=2, S=4096, D=512, H=8) on 8 Trainium2 cores.

Sharding: core c handles batch b = c//4 and heads {2*(c%4), 2*(c%4)+1}.

Fused single-pipeline design (v2): per query-chunk J the kernel runs
attention for head0 then head1 (k-major transposed scores, exp on ACT with
the padding mask folded into the per-partition bias), while the PE slack
under the ACT-bound steady state absorbs interleaved "filler" work: the
q/k/v projections for chunk J+1, the V transposes, and the q-major output
projection for chunk J-1.  Scores PSUM is double-buffered so ACT never
waits on QK; PV accumulates numerators + softmax denominator (ones column)
per head; the Pool engine moves PV results and output-projection tiles out
of PSUM (bf16), keeping DVE/ACT free.  Outputs are per-head undivided
projections po_h [S, 512] bf16 plus denominators; the host divides, sums
heads/cores, and adds bo.

PSUM map (8 banks): st 2 bufs x [128,1024]f32 (4) | pv [65,1024]f32 (2)
| aux 2 bufs x [128,512]f32 shared by proj pieces / V transposes / outproj.

Head x row-group layout: qT/kT keep head0 on partitions 0-63, head1 on
64-127; dupq/dupk hold the swapped copy so head h can issue even kb blocks
on PE row group 0 and odd kb blocks on row group 64 (concurrent tiles).
"""

import sys

sys.path.insert(0, "/opt/trn_rl_repo")

from contextlib import ExitStack

import ml_dtypes
import numpy as np

import concourse.bass as bass
import concourse.tile as tile
from concourse import bacc, bass_utils, mybir

B, S, D = 2, 4096, 512
H, HD = 8, 64
NCORES = 8
F32 = mybir.dt.float32
BF16 = mybir.dt.bfloat16
EXP = mybir.ActivationFunctionType.Exp
NPBF16 = ml_dtypes.bfloat16

CHUNK = 1024                  # query-chunk width
NCHUNK = S // CHUNK           # 4
KBLK = 128                    # key block (partition dim)
KB_PER_CHUNK = CHUNK // KBLK  # 8
NEG = -1.0e30


def _pieces(col0):
    """Split [col0, CHUNK) into <=512-wide pieces aligned to 512 boundaries."""
    out = []
    c = col0
    while c < CHUNK:
        nxt = min(CHUNK, (c // 512 + 1) * 512)
        out.append((c, nxt))
        c = nxt
    return out


class _Emitter:
    def __init__(self, nc, tc, ctx, io):
        self.nc = nc
        (self.xT, self.wq_p, self.wk_p, self.wv_p, self.wo01, self.bqkv,
         self.kbias, self.trimask, self.ident2, self.po0, self.po1,
         self.dens) = io

        const = ctx.enter_context(tc.tile_pool(name="const", bufs=1))
        self.sb = ctx.enter_context(tc.tile_pool(name="sb", bufs=1))
        self.etp = ctx.enter_context(tc.tile_pool(name="etp", bufs=6))
        self.xp = ctx.enter_context(tc.tile_pool(name="xp", bufs=2))
        self.ps_st = ctx.enter_context(
            tc.tile_pool(name="ps_st", bufs=2, space="PSUM"))
        self.ps_pv = ctx.enter_context(
            tc.tile_pool(name="ps_pv", bufs=1, space="PSUM"))
        self.ps_aux = ctx.enter_context(
            tc.tile_pool(name="ps_aux", bufs=2, space="PSUM"))

        # constants / weights
        self.wq_sb = const.tile([128, 512], BF16, tag="wq")
        self.wk_sb = const.tile([128, 512], BF16, tag="wk")
        self.wv_sb = const.tile([128, 512], BF16, tag="wv")
        self.wo_sb = const.tile([128, 512], BF16, tag="wo")
        self.bqkv_sb = const.tile([128, 3], F32, tag="bqkv")
        self.kbias_sb = const.tile([128, 32], F32, tag="kbias")
        self.tri_sb = const.tile([128, 128], BF16, tag="tri")
        self.id2_sb = const.tile([128, 64], BF16, tag="id2")
        onesf_sb = const.tile([128, 1], F32, tag="onesf")
        nc.vector.memset(onesf_sb[:], 1.0)
        for t, a in ((self.wq_sb, self.wq_p), (self.wk_sb, self.wk_p),
                     (self.wv_sb, self.wv_p), (self.bqkv_sb, self.bqkv),
                     (self.id2_sb, self.ident2), (self.kbias_sb, self.kbias),
                     (self.tri_sb, self.trimask), (self.wo_sb, self.wo01)):
            nc.sync.dma_start(t[:], a[:])

        # persistent intermediates
        self.qT = self.sb.tile([128, S], BF16, tag="qT")
        self.kT = self.sb.tile([128, S], BF16, tag="kT")
        self.dupq = self.sb.tile([128, S], BF16, tag="dupq")
        self.dupk = self.sb.tile([128, S], BF16, tag="dupk")
        self.v0 = self.sb.tile([128, 32 * 65], BF16, tag="v0")
        self.v1 = self.sb.tile([128, 32 * 65], BF16, tag="v1")
        self.oT01 = self.sb.tile([128, S], BF16, tag="oT01")
        self.den0 = self.sb.tile([1, S], F32, tag="den0")
        self.den1 = self.sb.tile([1, S], F32, tag="den1")
        for vdst in (self.v0, self.v1):
            ones_col = vdst.rearrange("p (k c) -> p k c", c=65)[:, :, 64:65]
            nc.vector.tensor_copy(
                ones_col, onesf_sb[:].to_broadcast((128, 32, 1)))

        self._fill = []

    # ---------------- filler units (emitted between attention steps) ----
    def fill(self, n=1):
        for _ in range(n):
            if self._fill:
                self._fill.pop(0)()

    def flush_fill(self):
        while self._fill:
            self._fill.pop(0)()

    def emit_dma_x(self, J):
        """Queue DMA of x chunk J. Returns the x tiles."""
        nc = self.nc
        x_sb = []
        for ks in range(4):
            xt = self.xp.tile([128, CHUNK], BF16, tag=f"x{ks}")
            x_sb.append(xt)

        def unit(ks=None):
            for ks in range(4):
                nc.sync.dma_start(
                    x_sb[ks][:],
                    self.xT[ks * 128:(ks + 1) * 128,
                            J * CHUNK:(J + 1) * CHUNK])
        return x_sb, unit

    def proj_units(self, J, x_sb):
        """q/k/v projections for chunk J + V transpose/repack + dup copies."""
        nc = self.nc
        units = []
        vT = self.sb.tile([128, CHUNK], BF16, tag="vTc", bufs=2)
        csl = slice(J * CHUNK, (J + 1) * CHUNK)

        for w_sb, bcol, dest, base in (
                (self.wq_sb, 0, self.qT, J * CHUNK),
                (self.wk_sb, 1, self.kT, J * CHUNK),
                (self.wv_sb, 2, vT, 0)):
            for half in range(2):
                lo = half * 512

                def unit(w_sb=w_sb, bcol=bcol, dest=dest, base=base, lo=lo):
                    ps = self.ps_aux.tile([128, 512], F32, tag="aux")
                    for ks in range(4):
                        nc.tensor.matmul(
                            ps[:],
                            w_sb[:, ks * 128:(ks + 1) * 128],
                            x_sb[ks][:, lo:lo + 512],
                            start=(ks == 0), stop=(ks == 3))
                    nc.vector.tensor_scalar_add(
                        dest[:, base + lo:base + lo + 512], ps[:],
                        self.bqkv_sb[:, bcol:bcol + 1])
                units.append(unit)

        # V -> k-major 65-stride blocks (per head)
        for hh, vdst in ((0, self.v0), (1, self.v1)):
            def unit(hh=hh, vdst=vdst):
                tr = self.ps_aux.tile([128, 512], BF16, tag="aux")
                for i in range(8):
                    nc.tensor.transpose(
                        tr[:, i * 64:(i + 1) * 64],
                        vT[hh * 64:(hh + 1) * 64, i * KBLK:(i + 1) * KBLK],
                        self.id2_sb[hh * 64:(hh + 1) * 64, :])
                dst = vdst[:, J * 8 * 65:(J + 1) * 8 * 65]
                dst = dst.rearrange("p (k c) -> p k c", c=65)[:, :, 0:64]
                nc.vector.tensor_copy(
                    dst, tr.rearrange("p (k c) -> p k c", c=64))
            units.append(unit)

        def dup_unit():
            # swapped copies: dupX rows 64-127 <- head0, rows 0-63 <- head1
            nc.vector.tensor_copy(self.dupq[64:128, csl], self.qT[0:64, csl])
            nc.vector.tensor_copy(self.dupq[0:64, csl], self.qT[64:128, csl])
            nc.vector.tensor_copy(self.dupk[64:128, csl], self.kT[0:64, csl])
            nc.vector.tensor_copy(self.dupk[0:64, csl], self.kT[64:128, csl])
        units.append(dup_unit)
        return units

    def ph3_units(self, J, hh):
        """Output projection for (chunk J, head hh), per 128-query block."""
        nc = self.nc
        units = []
        hsl = slice(hh * 64, (hh + 1) * 64)
        dram = self.po0 if hh == 0 else self.po1
        for qb in range(8):
            def unit(qb=qb):
                q0 = J * CHUNK + qb * 128
                po = self.ps_aux.tile([128, 512], F32, tag="aux")
                nc.tensor.matmul(
                    po[:], self.oT01[hsl, q0:q0 + 128],
                    self.wo_sb[hsl, :], start=True, stop=True)
                posb = self.sb.tile([128, 512], BF16, tag="posb", bufs=4)
                nc.vector.tensor_copy(posb[:], po[:])
                nc.sync.dma_start(dram[q0:q0 + 128, :], posb[:])
            units.append(unit)
        return units

    # ---------------- attention for (chunk J, head hh) -------------------
    def attention(self, J, hh):
        nc = self.nc
        # row-group assignment: "normal" tiles serve head hh at group
        # hh*64; the swapped dup tiles serve it at the other group.
        qn, kn = (self.qT, self.kT)
        qs, ks_ = (self.dupq, self.dupk)
        vsb = self.v0 if hh == 0 else self.v1
        den = self.den0 if hh == 0 else self.den1
        hs0 = hh * 64            # group of the natural layout
        hs1 = 64 - hs0           # group of the swapped layout

        pv = self.ps_pv.tile([65, CHUNK], F32, tag="pv")
        nkb = KB_PER_CHUNK * (J + 1)
        csl = slice(J * CHUNK, (J + 1) * CHUNK)

        def emit_qk(kb):
            p = kb - KB_PER_CHUNK * J
            col0 = KBLK * p if p >= 0 else 0
            if kb % 2 == 0:
                qsrc, ksrc, g = qn, kn, hs0
            else:
                qsrc, ksrc, g = qs, ks_, hs1
            gs = slice(g, g + 64)
            st = self.ps_st.tile([128, CHUNK], F32, tag="st")
            for pi, (a, b) in enumerate(_pieces(col0)):
                inst = nc.tensor.matmul(
                    st[:, a:b],
                    ksrc[gs, kb * KBLK:(kb + 1) * KBLK],
                    qsrc[gs, J * CHUNK + a:J * CHUNK + b],
                    start=True, stop=True)
                if pi > 0:  # same kT block already loaded as weights
                    inst.ins.ldweights = False
            return st

        # pipelined: QK for kb+1 issues (PE) before PV for kb, so the PE
        # keeps ACT fed while PV waits on the current exp.
        st = emit_qk(0)
        for kb in range(nkb):
            p = kb - KB_PER_CHUNK * J
            col0 = KBLK * p if p >= 0 else 0
            pieces = _pieces(col0)
            et = self.etp.tile([128, CHUNK], BF16, tag="et")
            nc.scalar.activation(
                et[:, col0:], st[:, col0:], EXP,
                bias=self.kbias_sb[:, kb:kb + 1], scale=0.125)
            if p >= 0:
                nc.vector.tensor_mul(
                    et[:, col0:col0 + KBLK], et[:, col0:col0 + KBLK],
                    self.tri_sb[:])
            if kb + 1 < nkb:
                st = emit_qk(kb + 1)
            self.fill(self._rate)
            for pi, (a, b) in enumerate(pieces):
                last_a = (kb == KB_PER_CHUNK * J + 3 and a < 512)
                last_b = (kb == nkb - 1)
                inst = nc.tensor.matmul(
                    pv[:, a:b],
                    vsb[:, kb * 65:(kb + 1) * 65],
                    et[:, a:b],
                    start=(kb == 0),
                    stop=(last_a if a < 512 else last_b))
                if pi > 0:  # same V block already loaded as weights
                    inst.ins.ldweights = False
        # move pv out (DVE; GPSIMD cannot read PSUM), freeing the pv slot
        hsl = slice(hh * 64, (hh + 1) * 64)
        nc.vector.tensor_copy(self.oT01[hsl, csl], pv[0:64, :])
        nc.vector.tensor_copy(den[:, csl], pv[64:65, :])

    def run(self):
        nc = self.nc
        # prologue: chunk 0 inputs + projections, serial
        x0, dma0 = self.emit_dma_x(0)
        dma0()
        for u in self.proj_units(0, x0):
            u()
        x1, dma1 = self.emit_dma_x(1)
        dma1()

        xs = {1: x1}
        for J in range(NCHUNK):
            fills_h0 = []
            if J + 1 < NCHUNK:
                fills_h0 += self.proj_units(J + 1, xs[J + 1])
            if J + 2 < NCHUNK:
                xn, dman = self.emit_dma_x(J + 2)
                xs[J + 2] = xn
                fills_h0.append(dman)
            if J > 0:
                fills_h0 += self.ph3_units(J - 1, 1)   # prev chunk, head1
            fills_h1 = self.ph3_units(J, 0)            # this chunk, head0
            for hh, fills in ((0, fills_h0), (1, fills_h1)):
                self._fill = fills
                niter = KB_PER_CHUNK * (J + 1)
                self._rate = (
                    max(1, (len(fills) + niter - 1) // niter) if fills else 0)
                self.attention(J, hh)
                self.flush_fill()

        for u in self.ph3_units(NCHUNK - 1, 1):
            u()
        nc.sync.dma_start(self.dens[0:1, :], self.den0[:])
        nc.sync.dma_start(self.dens[1:2, :], self.den1[:])


def _emit(nc, tc, ctx, io):
    _Emitter(nc, tc, ctx, io).run()


_CACHED = None


def _build():
    global _CACHED
    if _CACHED is not None:
        return _CACHED
    nc = bacc.Bacc("TRN2", target_bir_lowering=False, debug=False,
                   enable_asserts=False, num_devices=NCORES)
    names = [
        ("xT", [D, S], BF16), ("wq_p", [128, 512], BF16),
        ("wk_p", [128, 512], BF16), ("wv_p", [128, 512], BF16),
        ("wo01", [128, 512], BF16),
        ("bqkv", [128, 3], F32), ("kbias", [128, 32], F32),
        ("trimask", [128, 128], BF16), ("ident2", [128, 64], BF16),
    ]
    aps = [nc.dram_tensor(n, sh, dt_, kind="ExternalInput").ap()
           for n, sh, dt_ in names]
    po0 = nc.dram_tensor("po0", [S, D], BF16, kind="ExternalOutput").ap()
    po1 = nc.dram_tensor("po1", [S, D], BF16, kind="ExternalOutput").ap()
    dens = nc.dram_tensor("dens", [2, S], F32, kind="ExternalOutput").ap()
    with tile.TileContext(nc) as tc, ExitStack() as ctx:
        _emit(nc, tc, ctx, aps + [po0, po1, dens])
    nc.compile()
    _CACHED = nc
    return nc


def _host_inputs(x, attention_mask, Wq, bq, Wk, bk, Wv, bv, Wo, bo):
    f = np.float32
    x = np.asarray(x, f)
    mask = np.asarray(attention_mask)
    Wq, Wk, Wv, Wo = (np.asarray(w, f) for w in (Wq, Wk, Wv, Wo))
    bq, bk, bv = (np.asarray(b_, f) for b_ in (bq, bk, bv))
    tri = np.triu(np.ones((128, 128), NPBF16))      # [k,q]: 1 where q >= k
    id2 = np.tile(np.eye(64, dtype=NPBF16), (2, 1))
    in_maps = []
    for c in range(NCORES):
        b = c // 4
        h0 = 2 * (c % 4)
        hsl = slice(64 * h0, 64 * h0 + 128)

        def pack_w(W):
            wt = W[hsl, :].T                        # [512, 128] = Wh^T
            return np.ascontiguousarray(
                wt.reshape(4, 128, 128).transpose(1, 0, 2)
                .reshape(128, 512).astype(NPBF16))

        wo_t = Wo[:, hsl].T.astype(NPBF16)           # [128, 512]
        kb = np.where(mask[b] != 0, f(0.0), f(NEG)).astype(f)
        in_maps.append({
            "xT": np.ascontiguousarray(x[b].T.astype(NPBF16)),
            "wq_p": pack_w(Wq), "wk_p": pack_w(Wk), "wv_p": pack_w(Wv),
            "wo01": np.ascontiguousarray(wo_t),
            "bqkv": np.ascontiguousarray(
                np.stack([bq[hsl], bk[hsl], bv[hsl]], axis=1)),
            "kbias": np.ascontiguousarray(kb.reshape(32, 128).T),
            "trimask": tri, "ident2": id2,
        })
    return in_maps


def _assemble(results, bo):
    out = np.zeros((B, S, D), np.float32)
    for c in range(NCORES):
        r = results[c]
        dens = r["dens"]
        part = (r["po0"].astype(np.float32) / dens[0][:, None]
                + r["po1"].astype(np.float32) / dens[1][:, None])
        out[c // 4] += part
    out += np.asarray(bo, np.float32)
    return out


def kernel(**inputs) -> np.ndarray:
    nc = _build()
    in_maps = _host_inputs(**inputs)
    last_err = None
    for attempt in range(3):
        try:
            res = bass_utils.run_bass_kernel_spmd(
                nc, in_maps, core_ids=list(range(NCORES)))
            out = _assemble(res.results, inputs["bo"])
        except Exception as e:  # transient NRT/axon device errors
            last_err = e
            continue
        if np.isfinite(out).all():
            return out
        last_err = RuntimeError("non-finite output")
    raise last_err


def run_traced(inputs, **kwargs):
    """test.py helper: run with NTFF tracing, return (out, BassKernelResults)."""
    nc = _build()
    in_maps = _host_inputs(**inputs)
    res = bass_utils.run_bass_kernel_spmd(
        nc, in_maps, core_ids=list(range(NCORES)), trace=True, **kwargs)
    return _assemble(res.results, inputs["bo"]), res
